# revision 1
# baseline (speedup 1.0000x reference)
"""Trainium2 Bass kernel for nn_Attention_59339268161917.

Dense transformer attention layer (B=2, S=2048, DIM=2048, H=16, DH=128) with
RoPE, causal mask, and the reference's quirky output transpose:
    out = einsum('bhst,bhtd->bhsd', probs, v)           # [B,H,S,DH]
    out = out.transpose(0,1,3,2).reshape(B, S, DIM)     # rows = (h*DH+d), cols = s !
    y   = einsum('bsd,ed->bse', out, Wo)                # contraction over s

Sharding: 8 cores = (batch b in 0..1) x (head-group g in 0..3, 4 heads each).
Thanks to the quirky transpose, the final projection contracts over s with the
full Wo, so each core produces a DISJOINT row-slice y[b, 512g:512(g+1), :].
No collective / reduction needed; host concatenates.

Host preprocessing (= sharding-time layout choice): transposed x (xT [e,s]),
transposed+row-permuted W slices (rows deinterleaved per head: [evens; odds]
so RoPE operates on contiguous partition halves), transposed Wo, broadcast
cos/sin tables, and the 16 diagonal 128x128 mask tiles (pre-scaled by
sqrt(DH) so exp((raw + m*sqrt(DH)) / sqrt(DH)) == exp(raw/sqrt(DH) + m)).

Device pipeline per core (f32r matmuls, bf16 attention intermediates):
  A1) Q^T,K^T projections (Wq^T,Wk^T resident; xT streamed), RoPE fused into
      the PSUM->SBUF eviction on DVE.
  A2) V projection (Wv^T resident; xT streamed again).
  B)  Per (head, q-block of 128): scores matmul -> mask-add on diagonal tile
      -> exp with accumulated row-sum on ScalarE -> PE-transpose of exp'd
      probs tiles -> AV matmul accumulation -> 1/den normalize on eviction.
      Causal: strictly-upper blocks skipped (exp(-1e9)==0 exactly).
  C)  Output projection: Y[hd, e] accumulating over s-tiles, streaming Wo^T.
"""

import sys

sys.path.insert(0, "/opt/trn_rl_repo")

import numpy as np

B, S, DIM, H = 2, 2048, 2048, 16
DH = DIM // H          # 128
G = 4                  # head groups (cores per batch)
HPG = H // G           # heads per core = 4
J = HPG * DH           # per-core projection width = 512
NT = S // 128          # 16 s/t tiles
NE = DIM // 128        # 16 e tiles
SCALE = 1.0 / float(np.sqrt(DH))

_PROGRAMS = {}


def _build_program(causal: bool, phases: str = "ABC"):
    import concourse.bass as bass
    import concourse.mybir as mybir
    import concourse.tile as tile
    from concourse.masks import make_identity

    VW = J + HPG               # 516: per t-tile, 4 blocks of (128 V cols + 1 ones col)
    f32 = mybir.dt.float32
    f32r = mybir.dt.float32r
    bf16 = mybir.dt.bfloat16
    AF = mybir.ActivationFunctionType

    nc = bass.Bass(target_bir_lowering=False)

    # DRAM inputs (per-core shards, host-preprocessed layouts)
    xT = nc.dram_tensor("xT", [DIM, S], f32r, kind="ExternalInput")          # [e, s]
    wqT = nc.dram_tensor("wqT", [DIM, J], f32r, kind="ExternalInput")        # [e, j'] deinterleaved
    wkT = nc.dram_tensor("wkT", [DIM, J], f32r, kind="ExternalInput")
    wvT = nc.dram_tensor("wvT", [DIM, J], f32r, kind="ExternalInput")        # [e, d] original order
    woT = nc.dram_tensor("woT", [S, DIM], f32r, kind="ExternalInput")        # [s, e]
    cosb = nc.dram_tensor("cosb", [64, S], bf16, kind="ExternalInput")        # [freq, s]
    sinb = nc.dram_tensor("sinb", [64, S], bf16, kind="ExternalInput")
    # 16 diagonal 128x128 mask tiles (pre-scaled by sqrt(DH)), packed [128, 16*128]
    maskd = nc.dram_tensor("maskd", [128, NT * 128], bf16, kind="ExternalInput")
    y = nc.dram_tensor("y", [J, DIM], f32, kind="ExternalOutput")            # [hd, e]

    SC = 512                   # s-chunk for phase A
    NSC = S // SC              # 4

    with tile.TileContext(nc) as tc:
        with (
            tc.tile_pool(name="const", bufs=1) as constp,
            tc.tile_pool(name="qk", bufs=1) as qkp,
        ):
            # persistent activations (A..B): Q^T/K^T per head-tile [r;i] x s
            qT = qkp.tile([128, HPG * S], bf16, tag="qT")
            kT = qkp.tile([128, HPG * S], bf16, tag="kT")
            mask_sb = constp.tile([128, NT * 128], bf16, tag="mask")

            # =========== Phase A1: Q^T, K^T + RoPE ===========
            if "A" in phases:
             with (
                tc.tile_pool(name="a1w", bufs=1) as a1w,
                tc.tile_pool(name="a1x", bufs=2) as a1x,
                tc.tile_pool(name="rope", bufs=4) as ropep,
                tc.tile_pool(name="psA1", bufs=6, space="PSUM") as psA1,
            ):
                cos_sb = a1w.tile([64, S], bf16, tag="cos")
                sin_sb = a1w.tile([64, S], bf16, tag="sin")
                wq_sb = a1w.tile([128, NE, J], f32r, tag="wq")
                wk_sb = a1w.tile([128, NE, J], f32r, tag="wk")
                xTr = xT.rearrange("(ne p) s -> p ne s", p=128)
                xc_cache = {}
                xc0 = a1x.tile([128, NE, SC], f32r, tag="xc")
                # interleave weight and first-chunk quarters so the first
                # accumulation chain starts after one quarter of each
                for eq in range(4):
                    nc.sync.dma_start(
                        wq_sb[:, eq * 4:(eq + 1) * 4, :],
                        wqT.rearrange("(ne p) j -> p ne j", p=128)[:, eq * 4:(eq + 1) * 4, :])
                    nc.sync.dma_start(
                        xc0[:, eq * 4:(eq + 1) * 4, :],
                        xTr[:, eq * 4:(eq + 1) * 4, 0:SC])
                    nc.sync.dma_start(
                        wk_sb[:, eq * 4:(eq + 1) * 4, :],
                        wkT.rearrange("(ne p) j -> p ne j", p=128)[:, eq * 4:(eq + 1) * 4, :])
                nc.sync.dma_start(cos_sb[:], cosb[:])
                nc.sync.dma_start(sin_sb[:], sinb[:])
                nc.sync.dma_start(mask_sb[:], maskd[:])
                for sc in range(NSC):
                    if sc == 0:
                        xc = xc0
                    else:
                        xc = a1x.tile([128, NE, SC], f32r, tag="xc")
                        for eq in range(4):
                            nc.sync.dma_start(
                                xc[:, eq * 4:(eq + 1) * 4, :],
                                xTr[:, eq * 4:(eq + 1) * 4, sc * SC:(sc + 1) * SC],
                            )
                    if sc == NSC - 1:
                        xc_cache[sc] = xc
                    c2 = cos_sb[:, sc * SC:(sc + 1) * SC]
                    s2 = sin_sb[:, sc * SC:(sc + 1) * SC]
                    for w_sb, dstT in ((wq_sb, qT), (wk_sb, kT)):
                        for jt in range(HPG):
                            ps = psA1.tile([128, SC], f32, tag="ps_qk")
                            for et in range(NE):
                                nc.tensor.matmul(
                                    ps[:],
                                    w_sb[:, et, jt * 128:(jt + 1) * 128],
                                    xc[:, et, :],
                                    start=(et == 0), stop=(et == NE - 1),
                                )
                            # RoPE: rows 0:64 = r (even feats), 64:128 = i (odd)
                            dst = dstT[:, jt * S + sc * SC: jt * S + (sc + 1) * SC]
                            t1 = ropep.tile([64, SC], bf16, tag="t1")
                            t2 = ropep.tile([64, SC], bf16, tag="t2")
                            nc.vector.tensor_mul(t1[:], ps[:64, :], c2)
                            nc.vector.tensor_mul(t2[:], ps[64:, :], s2)
                            nc.gpsimd.tensor_sub(dst[:64, :], t1[:], t2[:])
                            t3 = ropep.tile([64, SC], bf16, tag="t1")
                            t4 = ropep.tile([64, SC], bf16, tag="t2")
                            nc.vector.tensor_mul(t3[:], ps[:64, :], s2)
                            nc.vector.tensor_mul(t4[:], ps[64:, :], c2)
                            nc.gpsimd.tensor_add(dst[64:, :], t3[:], t4[:])

                # ---- V projection (same scope; wv reuses the wq slot) ----
                # per t-tile: [h0 V(128) | 1 | h1 V(128) | 1 | ...] so a single
                # N=129 matmul accumulates AV and the softmax denominator
                vN = qkp.tile([128, NT * VW], bf16, tag="vN")
                nc.gpsimd.memset(vN[:], 1.0)
                wv_sb = a1w.tile([128, NE, J], f32r, tag="wq")
                nc.sync.dma_start(wv_sb[:], wvT.rearrange("(ne p) j -> p ne j", p=128))
                for sc in [NSC - 1] + list(range(NSC - 1)):
                    if sc in xc_cache:
                        xc = xc_cache[sc]
                    else:
                        xc = a1x.tile([128, NE, SC], f32r, tag="xc")
                        for eq in range(4):
                            nc.sync.dma_start(
                                xc[:, eq * 4:(eq + 1) * 4, :],
                                xTr[:, eq * 4:(eq + 1) * 4, sc * SC:(sc + 1) * SC],
                            )
                    for tt in range(SC // 128):
                        ps = psA1.tile([128, J], f32, tag="ps_qk")
                        for et in range(NE):
                            nc.tensor.matmul(
                                ps[:],
                                xc[:, et, tt * 128:(tt + 1) * 128],
                                wv_sb[:, et, :],
                                start=(et == 0), stop=(et == NE - 1),
                            )
                        gt = sc * (SC // 128) + tt
                        for hh in range(HPG):
                            nc.scalar.activation(
                                vN[:, gt * VW + hh * 129: gt * VW + hh * 129 + 128],
                                ps[:, hh * 128:(hh + 1) * 128], AF.Copy)

            # =========== Phase B: attention ===========
            if True:
             with tc.tile_pool(name="oN", bufs=1) as onp:
                # O natural: s-tile st -> [:, st*J:(st+1)*J] = [128 s, J hd]
                oN = onp.tile([128, NT * J], f32r, tag="oN")

                if "B" in phases:
                 with (
                    tc.tile_pool(name="att", bufs=3) as attp,
                    tc.tile_pool(name="egp", bufs=2) as egp,
                    tc.tile_pool(name="wo", bufs=2) as wop,
                    tc.tile_pool(name="psS", bufs=4, space="PSUM") as psS,
                    tc.tile_pool(name="psO", bufs=2, space="PSUM") as psO,
                    tc.tile_pool(name="psY", bufs=2, space="PSUM") as psY,
                ):
                    for ht in range(HPG):
                        qh = qT[:, ht * S:(ht + 1) * S]
                        kh = kT[:, ht * S:(ht + 1) * S]
                        for qg in range(NT // 4):
                            s0 = qg * 512
                            nk = (4 * qg + 4) if causal else NT
                            eg = egp.tile([128, NT * 512], bf16, tag="eg")
                            for kt in range(nk):
                                in_grp = (4 * qg) <= kt <= (4 * qg + 3)
                                off = (kt - 4 * qg) * 128 if (causal and in_grp) else 0
                                ps = psS.tile([128, 512], f32, tag="ps_s")
                                nc.tensor.matmul(
                                    ps[:, off:512],
                                    kh[:, kt * 128:(kt + 1) * 128],
                                    qh[:, s0 + off: s0 + 512],
                                    start=True, stop=True,
                                )
                                if in_grp:
                                    d = (kt - 4 * qg) * 128
                                    nc.vector.tensor_add(
                                        ps[:, d:d + 128], ps[:, d:d + 128],
                                        mask_sb[:, kt * 128:(kt + 1) * 128],
                                    )
                                nc.scalar.activation(
                                    eg[:, kt * 512 + off:(kt + 1) * 512],
                                    ps[:, off:512], AF.Exp, scale=SCALE,
                                )
                            for idx in range(4):
                                qb = 4 * qg + idx
                                nkt = qb + 1 if causal else NT
                                po = psO.tile([128, 129], f32, tag="ps_o")
                                for kt in range(nkt):
                                    nc.tensor.matmul(
                                        po[:],
                                        eg[:, kt * 512 + idx * 128: kt * 512 + idx * 128 + 128],
                                        vN[:, kt * VW + ht * 129: kt * VW + (ht + 1) * 129],
                                        start=(kt == 0), stop=(kt == nkt - 1),
                                    )
                                rec = attp.tile([128, 1], f32, tag="rec")
                                nc.vector.reciprocal(rec[:], po[:, 128:129])
                                nc.vector.tensor_scalar_mul(
                                    oN[:, qb * J + ht * 128: qb * J + (ht + 1) * 128],
                                    po[:, :128], rec[:],
                                )

                    # =========== Phase C: output projection ===========
                    EC = 512
                    if "C" in phases:
                      for ec in range(DIM // EC):
                         woc = wop.tile([128, NT, EC], f32r, tag="woc")
                         woTr = woT.rearrange("(nt p) e -> p nt e", p=128)
                         for sq in range(4):
                             nc.sync.dma_start(
                                 woc[:, sq * 4:(sq + 1) * 4, :],
                                 woTr[:, sq * 4:(sq + 1) * 4, ec * EC:(ec + 1) * EC],
                             )
                         for jt in range(HPG):
                             ps = psY.tile([128, EC], f32, tag="ps_y")
                             for st in range(NT):
                                 nc.tensor.matmul(
                                     ps[:],
                                     oN[:, st * J + jt * 128: st * J + (jt + 1) * 128],
                                     woc[:, st, :],
                                     start=(st == 0), stop=(st == NT - 1),
                                 )
                             ysb = attp.tile([128, EC], f32, tag="ysb")
                             nc.vector.tensor_copy(ysb[:], ps[:])
                             nc.sync.dma_start(
                                 y[jt * 128:(jt + 1) * 128, ec * EC:(ec + 1) * EC], ysb[:]
                             )

    import bass_rust
    bass_rust.move_matmul_waits_to_ldweights(nc.m)
    bass_rust.generate_event_semaphores(nc)
    return nc


def _get_program(causal: bool):
    if causal not in _PROGRAMS:
        _PROGRAMS[causal] = _build_program(causal)
    return _PROGRAMS[causal]


def _deinterleave_rows(w_slice):
    """Permute [128k, E] rows within each 128-row head block: evens then odds."""
    out = w_slice.reshape(-1, DH, w_slice.shape[-1])
    return np.concatenate([out[:, 0::2, :], out[:, 1::2, :]], axis=1).reshape(w_slice.shape)


def _is_causal_compatible(mask2d):
    causal_ref = np.triu(np.full((S, S), -1e9, dtype=np.float32), k=1)
    if np.array_equal(mask2d, causal_ref):
        return True
    # any mask that is 0 on/below the block sub-diagonal region outside the
    # diagonal tiles and <= -1e8 strictly above the diagonal tiles also works
    for i in range(NT):
        lo = mask2d[i * 128:(i + 1) * 128, : i * 128]
        if lo.size and not np.all(lo == 0.0):
            return False
        up = mask2d[i * 128:(i + 1) * 128, (i + 1) * 128:]
        if up.size and not np.all(up <= -1e8):
            return False
    return True


def _make_in_maps(inputs):
    x = np.asarray(inputs["x"], dtype=np.float32)
    Wq = np.asarray(inputs["Wq"], dtype=np.float32)
    Wk = np.asarray(inputs["Wk"], dtype=np.float32)
    Wv = np.asarray(inputs["Wv"], dtype=np.float32)
    Wo = np.asarray(inputs["Wo"], dtype=np.float32)
    freqs_cos = np.asarray(inputs["freqs_cos"], dtype=np.float32)
    freqs_sin = np.asarray(inputs["freqs_sin"], dtype=np.float32)
    mask2d = np.asarray(inputs["mask"], dtype=np.float32).reshape(S, S)

    import ml_dtypes
    cosb = np.ascontiguousarray(freqs_cos.T).astype(ml_dtypes.bfloat16)
    sinb = np.ascontiguousarray(freqs_sin.T).astype(ml_dtypes.bfloat16)
    maskd = np.concatenate(
        [mask2d[i * 128:(i + 1) * 128, i * 128:(i + 1) * 128].T for i in range(NT)], axis=1
    ) * np.float32(np.sqrt(DH))
    import ml_dtypes as _mld
    maskd = np.ascontiguousarray(maskd).astype(_mld.bfloat16)
    woT = np.ascontiguousarray(Wo.T)

    in_maps = []
    for c in range(8):
        b, g = divmod(c, G)
        rows = slice(g * J, (g + 1) * J)
        in_maps.append({
            "xT": np.ascontiguousarray(x[b].T),
            "wqT": np.ascontiguousarray(_deinterleave_rows(Wq[rows]).T),
            "wkT": np.ascontiguousarray(_deinterleave_rows(Wk[rows]).T),
            "wvT": np.ascontiguousarray(Wv[rows].T),
            "woT": woT,
            "cosb": cosb,
            "sinb": sinb,
            "maskd": maskd,
        })
    return in_maps


def _offdiag_tiles_zero(mask2d):
    m = mask2d.copy()
    for i in range(NT):
        m[i * 128:(i + 1) * 128, i * 128:(i + 1) * 128] = 0.0
    return bool(np.all(m == 0.0))


def _numpy_fallback(x, Wq, Wk, Wv, Wo, freqs_cos, freqs_sin, mask):
    q = (x @ Wq.T).reshape(B, S, H, DH)
    k = (x @ Wk.T).reshape(B, S, H, DH)
    v = (x @ Wv.T).reshape(B, S, H, DH)

    def rope(t):
        tr, ti = t[..., 0::2], t[..., 1::2]
        c = freqs_cos[None, :, None, :]
        s = freqs_sin[None, :, None, :]
        return np.stack([tr * c - ti * s, tr * s + ti * c], axis=-1).reshape(t.shape)

    q, k = rope(q), rope(k)
    q, k, v = (t.transpose(0, 2, 1, 3) for t in (q, k, v))
    m = mask.reshape(S, S)
    out = np.empty((B, H, S, DH), np.float32)
    for b in range(B):
        for h in range(H):
            sc = (q[b, h] @ k[b, h].T) / np.float32(np.sqrt(DH)) + m
            sc -= sc.max(axis=1, keepdims=True)
            e = np.exp(sc)
            out[b, h] = (e / e.sum(axis=1, keepdims=True)) @ v[b, h]
    out = out.transpose(0, 1, 3, 2).reshape(B, S, DIM)
    return (out @ Wo.T).astype(np.float32)


def kernel(x, Wq, Wk, Wv, Wo, freqs_cos, freqs_sin, mask):
    from concourse.bass_utils import run_bass_kernel_spmd

    inputs = {"x": x, "Wq": Wq, "Wk": Wk, "Wv": Wv, "Wo": Wo,
              "freqs_cos": freqs_cos, "freqs_sin": freqs_sin, "mask": mask}
    mask2d = np.asarray(mask, dtype=np.float32).reshape(S, S)
    causal = _is_causal_compatible(mask2d)
    if not causal and not _offdiag_tiles_zero(mask2d):
        return _numpy_fallback(
            np.asarray(x, np.float32), np.asarray(Wq, np.float32),
            np.asarray(Wk, np.float32), np.asarray(Wv, np.float32),
            np.asarray(Wo, np.float32), np.asarray(freqs_cos, np.float32),
            np.asarray(freqs_sin, np.float32), mask2d)
    nc = _get_program(causal)
    in_maps = _make_in_maps(inputs)

    res = run_bass_kernel_spmd(nc, in_maps, core_ids=list(range(8)))

    out = np.empty((B, S, DIM), dtype=np.float32)
    for c in range(8):
        b, g = divmod(c, G)
        out[b, g * J:(g + 1) * J, :] = res.results[c]["y"]
    return out



# revision 8
# speedup vs baseline: 1.0746x; 1.0746x over previous
"""Trainium2 Bass kernel for nn_Attention_59339268161917.

Dense transformer attention layer (B=2, S=2048, DIM=2048, H=16, DH=128) with
RoPE, causal mask, and the reference's quirky output transpose:
    out = einsum('bhst,bhtd->bhsd', probs, v)           # [B,H,S,DH]
    out = out.transpose(0,1,3,2).reshape(B, S, DIM)     # rows = (h*DH+d), cols = s !
    y   = einsum('bsd,ed->bse', out, Wo)                # contraction over s

Sharding: 8 cores = (batch b in 0..1) x (head-group g in 0..3, 4 heads each).
Thanks to the quirky transpose, the final projection contracts over s with the
full Wo, so each core produces a DISJOINT row-slice y[b, 512g:512(g+1), :].
No collective needed; host concatenates.

v2 design (per-head software pipeline, all-bf16 inputs):
  All inputs are cast to bf16 on the host (validated: rel err ~6e-3 vs 2e-2
  budget), halving DMA traffic.  x^T and the per-head packed W_{q,k,v} blocks
  are SBUF-resident; Wo^T streams per-head in 8 column chunks.

  The PE instruction stream interleaves, per head-slot h:
    A(h):   Q/K projections (+RoPE on DVE/Pool) and V projection per s-chunk
    B(h):   scores (PE) -> mask (DVE) -> exp (Act) -> AV+denominator (PE,
            ones-column trick) -> normalize (DVE), software-pipelined so the
            AV for query-group qg runs one iteration after its exp
    C(h-1): output projection chains for the previous head
  so the Activation engine's exp throughput (the old phase-B bottleneck) hides
  behind ~69us of independent PE work per slot, and the PE never starves.
"""

import sys

sys.path.insert(0, "/opt/trn_rl_repo")

import numpy as np

B, S, DIM, H = 2, 2048, 2048, 16
DH = DIM // H          # 128
G = 4                  # head groups (cores per batch)
HPG = H // G           # heads per core = 4
J = HPG * DH           # per-core output rows = 512
NT = S // 128          # 16 s/t tiles
NE = DIM // 128        # 16 e tiles
SC = 512               # s-chunk / q-group width
SCALE = 1.0 / float(np.sqrt(DH))
WB = 3 * DH            # packed q|k|v width per head = 384
EC = 256               # output-projection e-chunk
NEC = DIM // EC        # 8

_PROGRAMS = {}


def _build_program(causal: bool):
    import concourse.bass as bass
    import concourse.mybir as mybir
    import concourse.tile as tile

    f32 = mybir.dt.float32
    bf16 = mybir.dt.bfloat16
    AF = mybir.ActivationFunctionType

    nc = bass.Bass(target_bir_lowering=False)

    xbT = nc.dram_tensor("xbT", [DIM, S], bf16, kind="ExternalInput")        # x[b].T
    wqkv = nc.dram_tensor("wqkv", [DIM, HPG * WB], bf16, kind="ExternalInput")
    wob = nc.dram_tensor("wob", [S, DIM], bf16, kind="ExternalInput")        # Wo.T
    cosb = nc.dram_tensor("cosb", [64, S], bf16, kind="ExternalInput")
    sinb = nc.dram_tensor("sinb", [64, S], bf16, kind="ExternalInput")
    maskd = nc.dram_tensor("maskd", [128, NT * 128], bf16, kind="ExternalInput")
    y = nc.dram_tensor("y", [J, DIM], f32, kind="ExternalOutput")

    xbr = xbT.rearrange("(ne p) s -> p ne s", p=128)
    wqkvr = wqkv.rearrange("(ne p) j -> p ne j", p=128)
    wobr = wob.rearrange("(nt p) e -> p nt e", p=128)

    def nk_of(qg):
        return 4 * qg + 4 if causal else NT

    with tile.TileContext(nc) as tc:
        with (
            tc.tile_pool(name="const", bufs=1) as constp,
            tc.tile_pool(name="head", bufs=1) as headp,
            tc.tile_pool(name="von", bufs=2) as vonp,
            tc.tile_pool(name="eg", bufs=16) as egp,
            tc.tile_pool(name="wo", bufs=4) as wop,
            tc.tile_pool(name="rope", bufs=3) as ropep,
            tc.tile_pool(name="small", bufs=4) as smallp,
            tc.tile_pool(name="ysb", bufs=2) as ysbp,
            tc.tile_pool(name="psA", bufs=2, space="PSUM") as psA,
            tc.tile_pool(name="psV", bufs=1, space="PSUM") as psV,
            tc.tile_pool(name="psS", bufs=2, space="PSUM") as psS,
            tc.tile_pool(name="psO", bufs=2, space="PSUM") as psO,
            tc.tile_pool(name="psY", bufs=1, space="PSUM") as psY,
        ):
            xb_sb = constp.tile([128, NE, S], bf16, tag="xb")
            w_sb = constp.tile([128, NE, HPG * WB], bf16, tag="w")
            cos_sb = constp.tile([64, S], bf16, tag="cos")
            sin_sb = constp.tile([64, S], bf16, tag="sin")
            mask_sb = constp.tile([128, NT * 128], bf16, tag="mask")
            warm = constp.tile([128, SC], bf16, tag="warm")

            # ---- startup: small consts + PE warmup while wqkv/x stream in ----
            nc.sync.dma_start(cos_sb[:], cosb[:])
            nc.sync.dma_start(sin_sb[:], sinb[:])
            nc.sync.dma_start(mask_sb[:], maskd[:])
            nc.gpsimd.memset(warm[:], 0.0)
            for wch in range(2):
                pw = psA.tile([128, SC], f32, tag="psA")
                for i in range(12):
                    nc.tensor.matmul(
                        pw[:], warm[:, :128], warm[:],
                        start=(i == 0), stop=(i == 11), skip_group_check=True,
                    )
            for eq in range(4):
                nc.sync.dma_start(
                    w_sb[:, eq * 4:(eq + 1) * 4, 0:WB],
                    wqkvr[:, eq * 4:(eq + 1) * 4, 0:WB])
                nc.sync.dma_start(
                    xb_sb[:, eq * 4:(eq + 1) * 4, 0:SC],
                    xbr[:, eq * 4:(eq + 1) * 4, 0:SC])

            def dma_xchunk(sc):
                for eq in range(4):
                    nc.sync.dma_start(
                        xb_sb[:, eq * 4:(eq + 1) * 4, sc * SC:(sc + 1) * SC],
                        xbr[:, eq * 4:(eq + 1) * 4, sc * SC:(sc + 1) * SC])

            def dma_wslice(h):
                for eq in range(4):
                    nc.sync.dma_start(
                        w_sb[:, eq * 4:(eq + 1) * 4, h * WB:(h + 1) * WB],
                        wqkvr[:, eq * 4:(eq + 1) * 4, h * WB:(h + 1) * WB])

            # ---- helpers ----
            def rope_evict(ps, dstT, sc):
                c2 = cos_sb[:, sc * SC:(sc + 1) * SC]
                s2 = sin_sb[:, sc * SC:(sc + 1) * SC]
                dst = dstT[:, sc * SC:(sc + 1) * SC]
                t1 = ropep.tile([64, SC], bf16, tag="t1")
                t2 = ropep.tile([64, SC], bf16, tag="t2")
                nc.vector.tensor_mul(t1[:], ps[:64, :], c2)
                nc.vector.tensor_mul(t2[:], ps[64:, :], s2)
                nc.gpsimd.tensor_sub(dst[:64, :], t1[:], t2[:])
                t3 = ropep.tile([64, SC], bf16, tag="t1")
                t4 = ropep.tile([64, SC], bf16, tag="t2")
                nc.vector.tensor_mul(t3[:], ps[:64, :], s2)
                nc.vector.tensor_mul(t4[:], ps[64:, :], c2)
                nc.gpsimd.tensor_add(dst[64:, :], t3[:], t4[:])

            def proj_chain(h, sc, which, dstT):
                ps = psA.tile([128, SC], f32, tag="psA")
                base = h * WB + which * 128
                for et in range(NE):
                    nc.tensor.matmul(
                        ps[:], w_sb[:, et, base:base + 128],
                        xb_sb[:, et, sc * SC:(sc + 1) * SC],
                        start=(et == 0), stop=(et == NE - 1),
                        skip_group_check=True)
                rope_evict(ps, dstT, sc)

            def avp_units(h, sc, psv, vnh):
                units = []
                for tt in range(4):
                    tg = sc * 4 + tt
                    for eq in range(4):
                        def u(tt=tt, tg=tg, eq=eq):
                            for et in range(eq * 4, eq * 4 + 4):
                                nc.tensor.matmul(
                                    psv[:, tt, :],
                                    xb_sb[:, et, tg * 128:(tg + 1) * 128],
                                    w_sb[:, et, h * WB + 256:h * WB + WB],
                                    start=(et == 0), stop=(et == NE - 1),
                                    skip_group_check=True)
                        units.append(u)
                def ev():
                    nc.scalar.activation(
                        vnh[:, sc * 4:(sc + 1) * 4, 0:128], psv[:], AF.Copy)
                units.append(ev)
                return units

            def score_units(qh, kh, qg, eg_list):
                s0 = qg * SC
                units = []
                for kt in range(nk_of(qg)):
                    in_grp = (4 * qg) <= kt <= (4 * qg + 3)
                    off = (kt - 4 * qg) * 128 if (causal and in_grp) else 0
                    def u(kt=kt, off=off, in_grp=in_grp):
                        ps = psS.tile([128, SC], f32, tag="psS")
                        eg = egp.tile([128, SC], bf16, tag="eg")
                        nc.tensor.matmul(
                            ps[:, off:SC],
                            kh[:, kt * 128:(kt + 1) * 128],
                            qh[:, s0 + off:s0 + SC],
                            start=True, stop=True, skip_group_check=True)
                        if in_grp:
                            d = (kt - 4 * qg) * 128
                            nc.vector.tensor_add(
                                ps[:, d:d + 128], ps[:, d:d + 128],
                                mask_sb[:, kt * 128:(kt + 1) * 128])
                        nc.scalar.activation(
                            eg[:, off:SC], ps[:, off:SC], AF.Exp, scale=SCALE)
                        eg_list.append((kt, eg, off))
                    units.append(u)
                return units

            def emit_avo(qg0, eg_list, vnh, oNh):
                po2 = [psO.tile([128, 2, 129], f32, tag="po", name=f"po{i}")
                       for i in range(2)]
                po = [po2[i // 2][:, i % 2, :] for i in range(4)]
                started = [False] * 4
                last = [(4 * qg0 + i) if causal else (NT - 1) for i in range(4)]
                for (kt, eg, off) in eg_list:
                    for idx in range(4):
                        if causal and kt > 4 * qg0 + idx:
                            continue
                        if causal and off > idx * 128:
                            continue
                        nc.tensor.matmul(
                            po[idx][:],
                            eg[:, idx * 128:(idx + 1) * 128],
                            vnh[:, kt, :],
                            start=(not started[idx]), stop=(kt == last[idx]),
                            skip_group_check=True)
                        started[idx] = True
                for idx in range(4):
                    qb = 4 * qg0 + idx
                    rec = smallp.tile([128, 1], f32, tag="rec")
                    nc.vector.reciprocal(rec[:], po[idx][:, 128:129])
                    nc.vector.tensor_scalar_mul(
                        oNh[:, qb * 128:(qb + 1) * 128], po[idx][:, :128], rec[:])

            wo_q = []

            def dma_wo_chunk(ec):
                woc = wop.tile([128, NT, EC], bf16, tag="woc")
                nc.sync.dma_start(woc[:], wobr[:, :, ec * EC:(ec + 1) * EC])
                wo_q.append(woc)

            def c_units(hh, ec, oNh):
                woc = wo_q.pop(0)
                units = []
                box = [None]
                for eq in range(4):
                    def u(eq=eq):
                        if eq == 0:
                            box[0] = psY.tile([128, EC], f32, tag="psY",
                                              name="psy")
                        for st in range(eq * 4, eq * 4 + 4):
                            nc.tensor.matmul(
                                box[0][:], oNh[:, st * 128:(st + 1) * 128],
                                woc[:, st, :],
                                start=(st == 0), stop=(st == NT - 1),
                                skip_group_check=True)
                    units.append(u)
                def ev():
                    ysb = ysbp.tile([128, EC], f32, tag="ysb")
                    nc.vector.tensor_copy(ysb[:], box[0][:])
                    nc.sync.dma_start(
                        y[hh * 128:(hh + 1) * 128, ec * EC:(ec + 1) * EC], ysb[:])
                units.append(ev)
                return units

            def weave(score_us, filler_us):
                n = max(1, len(score_us))
                k = len(filler_us)
                fi = 0
                for i, su in enumerate(score_us):
                    su()
                    tgt = (i + 1) * k // n
                    while fi < tgt:
                        filler_us[fi]()
                        fi += 1
                while fi < k:
                    filler_us[fi]()
                    fi += 1

            # ---- head-slot pipeline ----
            vn_by_head = {}
            on_by_head = {}
            pending = None  # (qg0, eg_list, vnh, oNh)

            for h in range(HPG):
                qh = headp.tile([128, S], bf16, tag="qh")
                kh = headp.tile([128, S], bf16, tag="kh")
                vnh = vonp.tile([128, NT, 129], bf16, tag="vN")
                oNh = vonp.tile([128, S], bf16, tag="oN")
                nc.gpsimd.memset(vnh[:, :, 128:129], 1.0)
                vn_by_head[h] = vnh
                on_by_head[h] = oNh

                for qg in range(4):
                    # prefetch DMAs
                    if h == 0:
                        if qg < 3:
                            dma_xchunk(qg + 1)
                        if qg >= 1:
                            dma_wslice(qg)
                        if qg == 3:
                            dma_wo_chunk(0)
                            dma_wo_chunk(1)
                    else:
                        nch = 2 * qg + 2
                        for ec in (nch, nch + 1):
                            if ec < 8:
                                dma_wo_chunk(ec)
                            elif h < HPG - 1 or True:
                                dma_wo_chunk(ec - 8)  # next slot's chunks

                    proj_chain(h, qg, 0, qh)
                    proj_chain(h, qg, 1, kh)
                    if pending is not None:
                        emit_avo(*pending)
                        pending = None

                    psv = psV.tile([128, 4, 128], f32, tag="psV")
                    eg_list = []
                    sus = score_units(qh, kh, qg, eg_list)
                    fillers = avp_units(h, qg, psv, vnh)
                    if h >= 1:
                        fillers = fillers + c_units(h - 1, 2 * qg, on_by_head[h - 1])
                        fillers = fillers + c_units(h - 1, 2 * qg + 1, on_by_head[h - 1])
                    weave(sus, fillers)
                    pending = (qg, eg_list, vnh, oNh)

            # ---- tail slot: AV of last head's last group + C(3) ----
            emit_avo(*pending)
            for ec in range(8):
                if ec == 0:
                    dma_wo_chunk(2)
                    dma_wo_chunk(3)
                elif 4 <= ec + 3 <= 7:
                    dma_wo_chunk(ec + 3)
                for u in c_units(HPG - 1, ec, on_by_head[HPG - 1]):
                    u()

    import bass_rust
    bass_rust.move_matmul_waits_to_ldweights(nc.m)
    bass_rust.generate_event_semaphores(nc)
    return nc


def _get_program(causal: bool):
    if causal not in _PROGRAMS:
        _PROGRAMS[causal] = _build_program(causal)
    return _PROGRAMS[causal]


def _deinterleave_rows(w_slice):
    """Permute [128k, E] rows within each 128-row head block: evens then odds."""
    out = w_slice.reshape(-1, DH, w_slice.shape[-1])
    return np.concatenate([out[:, 0::2, :], out[:, 1::2, :]], axis=1).reshape(w_slice.shape)


def _is_causal_compatible(mask2d):
    causal_ref = np.triu(np.full((S, S), -1e9, dtype=np.float32), k=1)
    if np.array_equal(mask2d, causal_ref):
        return True
    for i in range(NT):
        lo = mask2d[i * 128:(i + 1) * 128, : i * 128]
        if lo.size and not np.all(lo == 0.0):
            return False
        up = mask2d[i * 128:(i + 1) * 128, (i + 1) * 128:]
        if up.size and not np.all(up <= -1e8):
            return False
    return True


def _make_in_maps(inputs):
    import ml_dtypes
    bf = ml_dtypes.bfloat16

    x = np.asarray(inputs["x"], dtype=np.float32)
    Wq = np.asarray(inputs["Wq"], dtype=np.float32)
    Wk = np.asarray(inputs["Wk"], dtype=np.float32)
    Wv = np.asarray(inputs["Wv"], dtype=np.float32)
    Wo = np.asarray(inputs["Wo"], dtype=np.float32)
    freqs_cos = np.asarray(inputs["freqs_cos"], dtype=np.float32)
    freqs_sin = np.asarray(inputs["freqs_sin"], dtype=np.float32)
    mask2d = np.asarray(inputs["mask"], dtype=np.float32).reshape(S, S)

    cosb = np.ascontiguousarray(freqs_cos.T).astype(bf)
    sinb = np.ascontiguousarray(freqs_sin.T).astype(bf)
    maskd = np.concatenate(
        [mask2d[i * 128:(i + 1) * 128, i * 128:(i + 1) * 128].T for i in range(NT)],
        axis=1) * np.float32(np.sqrt(DH))
    maskd = np.ascontiguousarray(maskd).astype(bf)
    wob = np.ascontiguousarray(Wo.T).astype(bf)

    in_maps = []
    for c in range(8):
        b, g = divmod(c, G)
        blocks = []
        for ht in range(HPG):
            rows = slice((g * HPG + ht) * DH, (g * HPG + ht + 1) * DH)
            blocks.append(_deinterleave_rows(Wq[rows]).T)
            blocks.append(_deinterleave_rows(Wk[rows]).T)
            blocks.append(Wv[rows].T)
        wqkv = np.ascontiguousarray(np.concatenate(blocks, axis=1)).astype(bf)
        in_maps.append({
            "xbT": np.ascontiguousarray(x[b].T).astype(bf),
            "wqkv": wqkv,
            "wob": wob,
            "cosb": cosb,
            "sinb": sinb,
            "maskd": maskd,
        })
    return in_maps


def _offdiag_tiles_zero(mask2d):
    m = mask2d.copy()
    for i in range(NT):
        m[i * 128:(i + 1) * 128, i * 128:(i + 1) * 128] = 0.0
    return bool(np.all(m == 0.0))


def _numpy_fallback(x, Wq, Wk, Wv, Wo, freqs_cos, freqs_sin, mask):
    q = (x @ Wq.T).reshape(B, S, H, DH)
    k = (x @ Wk.T).reshape(B, S, H, DH)
    v = (x @ Wv.T).reshape(B, S, H, DH)

    def rope(t):
        tr, ti = t[..., 0::2], t[..., 1::2]
        c = freqs_cos[None, :, None, :]
        s = freqs_sin[None, :, None, :]
        return np.stack([tr * c - ti * s, tr * s + ti * c], axis=-1).reshape(t.shape)

    q, k = rope(q), rope(k)
    q, k, v = (t.transpose(0, 2, 1, 3) for t in (q, k, v))
    m = mask.reshape(S, S)
    out = np.empty((B, H, S, DH), np.float32)
    for b in range(B):
        for h in range(H):
            sc = (q[b, h] @ k[b, h].T) / np.float32(np.sqrt(DH)) + m
            sc -= sc.max(axis=1, keepdims=True)
            e = np.exp(sc)
            out[b, h] = (e / e.sum(axis=1, keepdims=True)) @ v[b, h]
    out = out.transpose(0, 1, 3, 2).reshape(B, S, DIM)
    return (out @ Wo.T).astype(np.float32)


def kernel(x, Wq, Wk, Wv, Wo, freqs_cos, freqs_sin, mask):
    from concourse.bass_utils import run_bass_kernel_spmd

    inputs = {"x": x, "Wq": Wq, "Wk": Wk, "Wv": Wv, "Wo": Wo,
              "freqs_cos": freqs_cos, "freqs_sin": freqs_sin, "mask": mask}
    mask2d = np.asarray(mask, dtype=np.float32).reshape(S, S)
    causal = _is_causal_compatible(mask2d)
    if not causal and not _offdiag_tiles_zero(mask2d):
        return _numpy_fallback(
            np.asarray(x, np.float32), np.asarray(Wq, np.float32),
            np.asarray(Wk, np.float32), np.asarray(Wv, np.float32),
            np.asarray(Wo, np.float32), np.asarray(freqs_cos, np.float32),
            np.asarray(freqs_sin, np.float32), mask2d)
    nc = _get_program(causal)
    in_maps = _make_in_maps(inputs)

    res = run_bass_kernel_spmd(nc, in_maps, core_ids=list(range(8)))

    out = np.empty((B, S, DIM), dtype=np.float32)
    for c in range(8):
        b, g = divmod(c, G)
        out[b, g * J:(g + 1) * J, :] = res.results[c]["y"]
    return out
